# revision 21
# baseline (speedup 1.0000x reference)
"""Trainium2 Bass kernel for nn_BranchedNetwork (moe_routing).

Computation (reference):
    meas_embs = measurements @ W_meas + b_meas           [B, 512]
    embs      = concat([img_embs, meas_embs], axis=1)    [B, 1024]
    h_e       = relu(embs @ W1[e] + b1[e])               per expert e
    out_e     = h_e @ W2[e] + b2[e]
    p[i]      = out[command[i], i, 0]
    angle     = sigmoid(p) * 50 ; speed = clip(p, -1, 1)

Strategy:
  * Per-sample routing is done on the host: samples are grouped by
    command id, each group padded to a multiple of 8*128 rows and
    split evenly over the 8 cores (data parallel, weights replicated).
  * Only the selected expert runs per sample (4x less compute), and
    only column 0 of W2 is needed.
  * The measurement path is folded on the host:
      h_pre = img @ W1[e][:512] + meas @ (W_meas @ W1[e][512:])
              + (b_meas @ W1[e][512:] + b1[e])
    so the device contraction is K = 512 (img) + 8 (meas) + 1 (bias
    via a ones row) instead of 1024.
  * Device per 128-row tile: 5 PE matmuls (psum [128 rows, 512 hid]),
    ACT relu psum->sbuf, then one DVE tensor_tensor_reduce computing
    p = relu_h @ w2col + b2 as a fused multiply + free-dim reduction.
    Final sigmoid/scale/clip on [128, n_tiles] at full lane width.
"""

import os
import sys
import types

import numpy as np

if "/opt/trn_rl_repo" not in sys.path and not any(
    p.endswith("trn_rl_repo") for p in sys.path
):
    sys.path.insert(0, "/opt/trn_rl_repo")

B = 16384
EMB = 512
NUM_COMMANDS = 4
NUM_MEAS = 8
NCORES = 8
P = 128

# matmul dtype mode: "f32" (exact, 4 cyc/row), "f32r" (full speed,
# reduced internal precision), "bf16" (full speed + half DMA traffic)
MODE = os.environ.get("KERNEL_MM_MODE", "f32r")

_CACHE = {}


def _install_ntff_shim():
    """Recreate antenv.axon_hooks so trace=True works if requested."""
    if "antenv.axon_hooks" in sys.modules:
        return
    try:
        import antenv

        mod = types.ModuleType("antenv.axon_hooks")
        mod._hook = None
        mod.set_axon_ntff_profile_hook = lambda h: setattr(mod, "_hook", h)
        mod.get_axon_ntff_profile_hook = lambda: mod._hook
        sys.modules["antenv.axon_hooks"] = mod
        antenv.axon_hooks = mod
        from trn_agent_boot.trn_boot import _ntff_profile_via_ctypes

        mod.set_axon_ntff_profile_hook(
            _ntff_profile_via_ctypes("/opt/axon/libaxon_pjrt.so")
        )
    except Exception:
        pass


def _split_excess_waits(nc, max_waits=1):
    """The walrus in this container rejects instructions with more than
    one embedded sync-wait command. Waits execute in order on the
    issuing engine, so hoisting the excess onto preceding NOPs on the
    same engine is semantically identical."""
    from concourse import mybir

    n_split = 0
    for f in nc.m.functions:
        for bb in f.blocks:
            insts = list(bb.instructions)
            new_insts = []
            changed = False
            for inst in insts:
                si = inst.sync_info
                if si is not None and si.on_wait and len(si.on_wait) > max_waits:
                    waits = list(si.on_wait)
                    extra, keep = waits[:-max_waits], waits[-max_waits:]
                    while extra:
                        chunk, extra = extra[:max_waits], extra[max_waits:]
                        n_split += 1
                        nop = mybir.InstNoOp(
                            name=f"waitsplit_{n_split}_{inst.name}",
                            engine=inst.engine,
                            ins=[],
                            outs=[],
                            sync_info=mybir.SyncInfo(on_wait=chunk, on_update=[]),
                        )
                        new_insts.append(nop)
                    si.on_wait = keep
                    changed = True
                new_insts.append(inst)
            if changed:
                bb.instructions.clear()
                for i in new_insts:
                    bb.instructions.append(i)
    return n_split


def _strip_const_loads(nc):
    """Remove preamble loads of the const page when nothing reads it."""
    from concourse import mybir

    used = set()
    removed = 0
    for f in nc.m.functions:
        for bb in f.blocks:
            for inst in bb.instructions:
                for arg in list(inst.ins):
                    t = getattr(getattr(arg, "bass_ap", None), "tensor", None)
                    n = getattr(t, "name", "") or ""
                    if n.startswith("const-"):
                        used.add(n)
    if used:
        return 0
    for f in nc.m.functions:
        for bb in f.blocks:
            keep = []
            for inst in bb.instructions:
                if type(inst).__name__ == "InstTensorLoad":
                    outs = list(inst.outs)
                    names = []
                    for a in outs:
                        t = getattr(getattr(a, "bass_ap", None), "tensor", None)
                        names.append(getattr(t, "name", "") or "")
                    if names and all(n.startswith("const-") for n in names):
                        removed += 1
                        continue
                keep.append(inst)
            if len(keep) != len(bb.instructions):
                bb.instructions.clear()
                for i in keep:
                    bb.instructions.append(i)
    return removed


def _strip_tail(nc):
    """Remove the end-of-kernel barrier/sem-reset tail.

    The runtime clears semaphores in its own exec preamble, and every
    engine's results flow into the output DMA via data-dependency
    semaphores, so the only thing that must remain is the sync-engine
    DRAIN that flushes the output DMA queue."""
    from concourse import mybir

    f = nc.m.functions[0]
    bb = f.blocks[-1]
    insts = list(bb.instructions)
    idx = None
    for i, inst in enumerate(insts):
        if isinstance(inst, mybir.InstDrain) and inst.engine == mybir.EngineType.SP:
            idx = i
            break
    if idx is None:
        return 0
    kept = insts[: idx + 1]
    drain = kept[-1]
    if drain.sync_info is not None:
        drain.sync_info.on_wait = []
    removed = len(insts) - len(kept)
    bb.instructions.clear()
    for i in kept:
        bb.instructions.append(i)
    return removed


def _np_sto_dtype(mode):
    if mode == "bf16":
        import ml_dtypes

        return ml_dtypes.bfloat16
    return np.float32


def _route(command):
    """Group sample indices by expert, pad each group to a multiple of
    8*128 and split evenly across cores.

    Returns caps [E] (rows per core per expert) and I [NCORES, R] row
    index arrays (R = sum(caps))."""
    caps = []
    parts = []  # per expert: [NCORES, cap_e] padded index array
    for e in range(NUM_COMMANDS):
        idx = np.nonzero(command == e)[0].astype(np.int64)
        n = len(idx)
        cap = int(np.ceil(n / (NCORES * P))) * P if n else 0
        caps.append(cap)
        if cap == 0:
            parts.append(np.zeros((NCORES, 0), np.int64))
            continue
        pad = NCORES * cap - n
        idx_pad = np.concatenate([idx, np.full(pad, idx[-1], np.int64)])
        parts.append(idx_pad.reshape(NCORES, cap))
    desc = sorted(range(NUM_COMMANDS), key=lambda e: -caps[e])
    # small expert first (fast DMA start), small expert last (short tail)
    order = [desc[2], desc[0], desc[1], desc[3]]
    I = [np.concatenate([parts[e][k] for e in order]) for k in range(NCORES)]
    return [caps[e] for e in order], order, np.stack(I)


def _build_program(R, caps, eorder, b2c, n_pos, mode):
    from contextlib import ExitStack

    import concourse.bass as bass
    import concourse.tile as tile
    from concourse import mybir

    f32 = mybir.dt.float32
    # matmul-operand dtype (the whole producer chain must carry it for
    # the fp32r BIR verifier) and elementwise/storage dtype
    if mode == "bf16":
        MMD = mybir.dt.bfloat16
        STO = mybir.dt.bfloat16
    elif mode == "f32r":
        MMD = mybir.dt.float32r
        STO = f32
    else:
        MMD = f32
        STO = f32
    T = R // P

    pack = os.environ.get("KERNEL_PACK_MEAS", "1") == "1"
    nc = bass.Bass()
    # all arrays are PRE-TILED on the host so every DMA is a dense
    # [partition, contiguous-bytes] copy (cheap descriptor generation)
    imgT_d = nc.declare_dram_parameter("img_pre", [P, 4 * R], MMD, isOutput=False)
    if pack:
        measT_d = nc.declare_dram_parameter("measRep", [P, R], MMD, isOutput=False)
    else:
        measT_d = nc.declare_dram_parameter(
            "measAug", [NUM_MEAS + 1, R], MMD, isOutput=False
        )
    A_d = nc.declare_dram_parameter("A_pre", [NUM_COMMANDS, P, 4 * EMB], MMD, isOutput=False)
    if pack:
        WfAug_d = nc.declare_dram_parameter(
            "WfAugRep", [P, NUM_COMMANDS, EMB], MMD, isOutput=False
        )
    else:
        WfAug_d = nc.declare_dram_parameter(
            "WfAug_pre", [NUM_MEAS + 1, NUM_COMMANDS, EMB], MMD, isOutput=False
        )
    b2tail_d = nc.declare_dram_parameter("b2tail", [P, T], f32, isOutput=False)
    outp_d = nc.declare_dram_parameter("outp", [P, 2, T], f32, isOutput=True)

    with tile.TileContext(nc) as tc:
        with ExitStack() as ctx:
            const_pool = ctx.enter_context(tc.tile_pool(name="const", bufs=1))
            w_pool = ctx.enter_context(tc.tile_pool(name="w", bufs=16))
            img_pool = ctx.enter_context(tc.tile_pool(name="img", bufs=16))
            junk_pool = ctx.enter_context(tc.tile_pool(name="junk", bufs=4))
            out_pool = ctx.enter_context(tc.tile_pool(name="out", bufs=1))
            ps_pool = ctx.enter_context(tc.tile_pool(name="ps", bufs=6, space="PSUM"))
            psw_pool = ctx.enter_context(tc.tile_pool(name="psw", bufs=1, space="PSUM"))

            # greedy least-loaded DMA queue assignment over the three
            # DMA-capable engines (SP + ACT hwdge, Pool swdge), with
            # transfers grouped into waves chained by semaphores so the
            # first-needed expert's data doesn't share DMA bandwidth
            # with later experts' transfers
            from concourse.tile_rust import add_dep_helper

            dma_engines = [nc.sync, nc.scalar, nc.gpsimd]
            # measured queue service rates differ: sync-HW ~1.4x scalar-HW,
            # gpsimd-SW slightly slower; balance by completion time
            dma_speed = [1.4, 1.0, 0.92]
            dma_load = [0.0, 0.0, 0.0]
            waves = [[]]

            def dma(dst, src, nbytes):
                qi = dma_load.index(min(dma_load))
                dma_load[qi] += nbytes / dma_speed[qi]
                inst = dma_engines[qi].dma_start(dst, src)
                waves[-1].append(inst)
                return inst

            def next_wave():
                if waves[-1]:
                    waves.append([])

            esz = 2 if mode == "bf16" else 4
            mrows = P if pack else NUM_MEAS + 1
            measT_sb = const_pool.tile([mrows, R], MMD)
            dma(measT_sb[:], measT_d[:], mrows * R * esz)
            WfAug_sb = const_pool.tile([mrows, NUM_COMMANDS, EMB], MMD)
            dma(WfAug_sb[:], WfAug_d[:], mrows * 4 * EMB * esz)
            b2tail_sb = const_pool.tile([P, T], f32, tag="b2tail", name="b2tail_sb")
            dma(b2tail_sb[:], b2tail_d[:], P * T * 4)
            zbias = const_pool.tile([P, 1], f32)
            nc.vector.memset(zbias[:], 0.0)
            p_pos = {}
            p_neg = {}
            for i, cap in enumerate(caps):
                if cap == 0:
                    continue
                tseg = cap // P
                p_pos[i] = out_pool.tile([P, tseg], f32, tag=f"pp_{i}", name=f"pp_{i}")
                p_neg[i] = out_pool.tile([P, tseg], f32, tag=f"pn_{i}", name=f"pn_{i}")
                nc.vector.memset(p_pos[i][:], 0.0)
                nc.vector.memset(p_neg[i][:], 0.0)

            A_sb = {}
            img_sb = {}
            for i, cap in enumerate(caps):
                if cap == 0:
                    continue
                next_wave()
                e = eorder[i]
                base = 4 * sum(caps[:i])
                for c in range(4):
                    A_sb[i, c] = w_pool.tile(
                        [P, EMB], MMD, tag="A", name=f"A_sb_{i}_{c}"
                    )
                    dma(
                        A_sb[i, c][:],
                        A_d[e][:, c * EMB : (c + 1) * EMB],
                        P * EMB * esz,
                    )
                    img_sb[i, c] = img_pool.tile(
                        [P, cap], MMD, tag="img", name=f"img_sb_{i}_{c}"
                    )
                    dma(
                        img_sb[i, c][:],
                        imgT_d[:, base + c * cap : base + (c + 1) * cap],
                        P * cap * esz,
                    )

            if os.environ.get("KERNEL_WAVES", "0") == "1":
                for k in range(1, len(waves)):
                    gate = waves[k - 1][-1]
                    for inst in waves[k]:
                        add_dep_helper(
                            inst.ins, gate.ins, sync=True, reason="dma wave chaining"
                        )

            # keep the PE busy during the initial DMA window so the HAM
            # clock gate is warm when real matmuls start
            warm_a = const_pool.tile([P, EMB], MMD, tag="warm_a", name="warm_a")
            nc.vector.memset(warm_a[:], 0.0)
            ps_w = psw_pool.tile([P, EMB], f32, tag="warm_ps", name="ps_warm")
            N_WARM = 16
            for w in range(N_WARM):
                nc.tensor.matmul(
                    ps_w[:],
                    lhsT=warm_a[:, :P],
                    rhs=warm_a[:],
                    start=(w == 0),
                    stop=(w == N_WARM - 1),
                )
            junkw = junk_pool.tile([P, EMB], STO, tag="junk")
            nc.scalar.activation(
                junkw[:], ps_w[:], mybir.ActivationFunctionType.Copy
            )

            for i, cap in enumerate(caps):
                e = eorder[i]
                off = sum(caps[:i])
                nt = cap // P
                group, ps_of = {}, {}
                for r0 in range(0, nt, 4):
                    group[r0] = list(range(r0, min(r0 + 4, nt)))
                for r in range(nt):
                    if pack and r in group:
                        # emit the whole group's img matmuls, then the
                        # packed meas matmuls as concurrent row-groups
                        for rr in group[r]:
                            psr = ps_pool.tile(
                                [P, EMB], f32, tag="h", name=f"ps_{i}_{rr}"
                            )
                            ps_of[rr] = psr
                            for ko in range(4):
                                nc.tensor.matmul(
                                    psr[:],
                                    lhsT=img_sb[i, ko][:, rr * P : (rr + 1) * P],
                                    rhs=A_sb[i, ko][:],
                                    start=(ko == 0),
                                    stop=False,
                                )
                        for j, rr in enumerate(group[r]):
                            col = off + rr * P
                            nc.tensor.matmul(
                                ps_of[rr][:],
                                lhsT=measT_sb[
                                    32 * j : 32 * j + NUM_MEAS + 1, col : col + P
                                ],
                                rhs=WfAug_sb[32 * j : 32 * j + NUM_MEAS + 1, e, :],
                                start=False,
                                stop=True,
                                tile_position=(32 * j, 0),
                            )
                    if pack:
                        ps = ps_of[r]
                    else:
                        ps = ps_pool.tile([P, EMB], f32, tag="h")
                        for ko in range(4):
                            nc.tensor.matmul(
                                ps[:],
                                lhsT=img_sb[i, ko][:, r * P : (r + 1) * P],
                                rhs=A_sb[i, ko][:],
                                start=(ko == 0),
                                stop=False,
                            )
                        col = off + r * P
                        nc.tensor.matmul(
                            ps[:],
                            lhsT=measT_sb[:, col : col + P],
                            rhs=WfAug_sb[:, e, :],
                            start=False,
                            stop=True,
                        )
                    junk = junk_pool.tile([P, EMB], STO, tag="junk")
                    npe = n_pos[e]
                    if npe > 0:
                        nc.scalar.activation(
                            junk[:, :npe],
                            ps[:, :npe],
                            mybir.ActivationFunctionType.Relu,
                            bias=zbias[:],
                            accum_out=p_pos[i][:, r : r + 1],
                        )
                    if npe < EMB:
                        junk2 = junk_pool.tile([P, EMB], STO, tag="junk2")
                        nc.vector.tensor_scalar(
                            junk2[:, npe:],
                            ps[:, npe:],
                            0.0,
                            0.0,
                            mybir.AluOpType.max,
                            mybir.AluOpType.add,
                            accum_out=p_neg[i][:, r : r + 1],
                        )

                tseg = cap // P
                seg = slice(off // P, off // P + tseg)
                q = out_pool.tile([P, tseg], f32, tag=f"q_{i}", name=f"q_{i}")
                sig = out_pool.tile([P, tseg], f32, tag=f"sig_{i}", name=f"sig_{i}")
                outs = out_pool.tile(
                    [P, 2, tseg], f32, tag=f"outs_{i}", name=f"outs_{i}"
                )
                nc.vector.tensor_tensor(
                    q[:], p_pos[i][:], p_neg[i][:], mybir.AluOpType.subtract
                )
                nc.vector.tensor_add(q[:], q[:], b2tail_sb[:, seg])
                nc.scalar.activation(
                    sig[:],
                    q[:],
                    mybir.ActivationFunctionType.Sigmoid,
                    bias=zbias[:],
                )
                nc.vector.tensor_scalar_mul(outs[:, 0, :], sig[:], 50.0)
                nc.vector.tensor_scalar(
                    outs[:, 1, :],
                    q[:],
                    1.0,
                    -1.0,
                    mybir.AluOpType.min,
                    mybir.AluOpType.max,
                )
                dma(outp_d[:, :, seg], outs[:], P * 2 * tseg * 4)



    _strip_tail(nc)
    _split_excess_waits(nc)
    return nc


def _prepare(inputs, mode):
    img_embs = np.asarray(inputs["img_embs"], np.float32)
    measurements = np.asarray(inputs["measurements"], np.float32)
    command = np.asarray(inputs["command"])
    W_meas = np.asarray(inputs["W_meas"], np.float32)
    b_meas = np.asarray(inputs["b_meas"], np.float32)
    W1 = np.asarray(inputs["W1"], np.float32)
    b1 = np.asarray(inputs["b1"], np.float32)
    W2 = np.asarray(inputs["W2"], np.float32)
    b2 = np.asarray(inputs["b2"], np.float32)

    sto = _np_sto_dtype(mode)
    caps, eorder, I = _route(command)
    R = int(sum(caps))

    # fold measurement path (float64 for the host-side precompute)
    W1h = W1[:, EMB:, :].astype(np.float64)
    Wf = np.einsum("md,edh->emh", W_meas.astype(np.float64), W1h)
    b_eff = np.einsum("d,edh->eh", b_meas.astype(np.float64), W1h) + b1
    A64 = W1[:, :EMB, :].astype(np.float64)

    # fold |w2[:, 0]| into the hidden columns and permute them so the
    # w2>0 columns come first: p = sum(relu(pos cols)) - sum(relu(neg
    # cols)), computed for free by the ACT accum during the relu pass.
    w2c = W2[:, :, 0].astype(np.float64)
    n_pos = []
    A_s = np.empty_like(A64)
    Wf_s = np.empty_like(Wf)
    b_eff_s = np.empty_like(b_eff)
    for e in range(NUM_COMMANDS):
        perm = np.argsort(w2c[e] <= 0, kind="stable")
        n_pos.append(int((w2c[e] > 0).sum()))
        sc = np.abs(w2c[e])[perm]
        A_s[e] = A64[e][:, perm] * sc[None, :]
        Wf_s[e] = Wf[e][:, perm] * sc[None, :]
        b_eff_s[e] = b_eff[e][perm] * sc
    WfAug = np.concatenate([Wf_s, b_eff_s[:, None, :]], axis=1).astype(sto)
    A = np.ascontiguousarray(A_s).astype(sto)  # [E,512,512]
    b2c = [float(x) for x in b2[:, 0]]

    T = R // P
    col_expert = np.concatenate(
        [np.full(caps[i] // P, eorder[i], np.int64) for i in range(NUM_COMMANDS)]
    )
    b2tail = np.broadcast_to(
        np.array([b2c[e] for e in col_expert], np.float32)[None, :], (P, T)
    ).copy()

    # pre-tiled shared weights: every device DMA is a dense 2D copy
    A_pre = np.ascontiguousarray(
        A.reshape(NUM_COMMANDS, 4, P, EMB).transpose(0, 2, 1, 3).reshape(
            NUM_COMMANDS, P, 4 * EMB
        )
    )
    WfAug_pre = np.ascontiguousarray(WfAug.transpose(1, 0, 2))  # [9, E, 512]

    pack = os.environ.get("KERNEL_PACK_MEAS", "1") == "1"
    if pack:
        # replicate WfAug at partition offsets 0/32/64/96 for the
        # row-group-packed meas matmuls
        WfAugRep = np.zeros((P, NUM_COMMANDS, EMB), WfAug_pre.dtype)
        for j in range(4):
            WfAugRep[32 * j : 32 * j + NUM_MEAS + 1] = WfAug_pre
        WfAugRep = np.ascontiguousarray(WfAugRep)

    imgT = img_embs.T.astype(sto)  # [512, B] cast once
    measT = measurements.T  # [8, B]
    ones_row = np.ones((1, R), np.float32).astype(sto)
    in_maps = []
    for k in range(NCORES):
        Ik = I[k]
        imgT_k = imgT[:, Ik].reshape(4, P, R)  # [o, p, r]
        img_pre = np.concatenate(
            [
                imgT_k[:, :, sum(caps[:e]) : sum(caps[: e + 1])]
                .transpose(1, 0, 2)
                .reshape(P, 4 * caps[e])
                for e in range(NUM_COMMANDS)
                if caps[e]
            ],
            axis=1,
        )
        measAug_k = np.concatenate(
            [measT[:, Ik].astype(sto), ones_row], axis=0
        )
        if pack:
            measRep_k = np.zeros((P, R), measAug_k.dtype)
            for j in range(4):
                measRep_k[32 * j : 32 * j + NUM_MEAS + 1] = measAug_k
            m = {
                "img_pre": np.ascontiguousarray(img_pre),
                "measRep": np.ascontiguousarray(measRep_k),
                "A_pre": A_pre,
                "WfAugRep": WfAugRep,
                "b2tail": b2tail,
            }
        else:
            m = {
                "img_pre": np.ascontiguousarray(img_pre),
                "measAug": measAug_k,
                "A_pre": A_pre,
                "WfAug_pre": WfAug_pre,
                "b2tail": b2tail,
            }
        in_maps.append(m)
    return in_maps, I, R, caps, eorder, b2c, n_pos


def _run(inputs, mode=None, trace=False):
    """Returns ((angle, speed), BassKernelResults)."""
    mode = mode or MODE
    _install_ntff_shim()
    from concourse.bass_utils import run_bass_kernel_spmd

    in_maps, I, R, caps, eorder, b2c, n_pos = _prepare(inputs, mode)
    key = (
        R,
        tuple(caps),
        tuple(eorder),
        mode,
        tuple(np.float32(b) for b in b2c),
        tuple(n_pos),
    )
    if key not in _CACHE:
        _CACHE[key] = _build_program(R, caps, eorder, b2c, n_pos, mode)
    nc = _CACHE[key]

    res = run_bass_kernel_spmd(
        nc, in_maps, core_ids=list(range(NCORES)), trace=trace
    )

    angle = np.zeros(B, np.float32)
    speed = np.zeros(B, np.float32)
    for k in range(NCORES):
        outp = res.results[k]["outp"]  # [128, 2, T]
        Ik = I[k]
        angle[Ik] = outp[:, 0, :].T.reshape(R)
        speed[Ik] = outp[:, 1, :].T.reshape(R)
    return (angle, speed), res


def kernel(**inputs):
    out, _ = _run(inputs)
    return out


# revision 22
# speedup vs baseline: 1.0376x; 1.0376x over previous
"""Trainium2 Bass kernel for nn_BranchedNetwork (moe_routing).

Computation (reference):
    meas_embs = measurements @ W_meas + b_meas           [B, 512]
    embs      = concat([img_embs, meas_embs], axis=1)    [B, 1024]
    h_e       = relu(embs @ W1[e] + b1[e])               per expert e
    out_e     = h_e @ W2[e] + b2[e]
    p[i]      = out[command[i], i, 0]
    angle     = sigmoid(p) * 50 ; speed = clip(p, -1, 1)

Strategy:
  * Per-sample routing is done on the host: samples are grouped by
    command id, each group padded to a multiple of 8*128 rows and
    split evenly over the 8 cores (data parallel, weights replicated).
  * Only the selected expert runs per sample (4x less compute), and
    only column 0 of W2 is needed.
  * The measurement path is folded on the host:
      h_pre = img @ W1[e][:512] + meas @ (W_meas @ W1[e][512:])
              + (b_meas @ W1[e][512:] + b1[e])
    so the device contraction is K = 512 (img) + 8 (meas) + 1 (bias
    via a ones row) instead of 1024.
  * Device per 128-row tile: 5 PE matmuls (psum [128 rows, 512 hid]),
    ACT relu psum->sbuf, then one DVE tensor_tensor_reduce computing
    p = relu_h @ w2col + b2 as a fused multiply + free-dim reduction.
    Final sigmoid/scale/clip on [128, n_tiles] at full lane width.
"""

import os
import sys
import types

import numpy as np

if "/opt/trn_rl_repo" not in sys.path and not any(
    p.endswith("trn_rl_repo") for p in sys.path
):
    sys.path.insert(0, "/opt/trn_rl_repo")

B = 16384
EMB = 512
NUM_COMMANDS = 4
NUM_MEAS = 8
NCORES = 8
P = 128

# matmul dtype mode: "f32" (exact, 4 cyc/row), "f32r" (full speed,
# reduced internal precision), "bf16" (full speed + half DMA traffic)
MODE = os.environ.get("KERNEL_MM_MODE", "f32r")

_CACHE = {}


def _install_ntff_shim():
    """Recreate antenv.axon_hooks so trace=True works if requested."""
    if "antenv.axon_hooks" in sys.modules:
        return
    try:
        import antenv

        mod = types.ModuleType("antenv.axon_hooks")
        mod._hook = None
        mod.set_axon_ntff_profile_hook = lambda h: setattr(mod, "_hook", h)
        mod.get_axon_ntff_profile_hook = lambda: mod._hook
        sys.modules["antenv.axon_hooks"] = mod
        antenv.axon_hooks = mod
        from trn_agent_boot.trn_boot import _ntff_profile_via_ctypes

        mod.set_axon_ntff_profile_hook(
            _ntff_profile_via_ctypes("/opt/axon/libaxon_pjrt.so")
        )
    except Exception:
        pass


def _split_excess_waits(nc, max_waits=1):
    """The walrus in this container rejects instructions with more than
    one embedded sync-wait command. Waits execute in order on the
    issuing engine, so hoisting the excess onto preceding NOPs on the
    same engine is semantically identical."""
    from concourse import mybir

    n_split = 0
    for f in nc.m.functions:
        for bb in f.blocks:
            insts = list(bb.instructions)
            new_insts = []
            changed = False
            for inst in insts:
                si = inst.sync_info
                if si is not None and si.on_wait and len(si.on_wait) > max_waits:
                    waits = list(si.on_wait)
                    extra, keep = waits[:-max_waits], waits[-max_waits:]
                    while extra:
                        chunk, extra = extra[:max_waits], extra[max_waits:]
                        n_split += 1
                        nop = mybir.InstNoOp(
                            name=f"waitsplit_{n_split}_{inst.name}",
                            engine=inst.engine,
                            ins=[],
                            outs=[],
                            sync_info=mybir.SyncInfo(on_wait=chunk, on_update=[]),
                        )
                        new_insts.append(nop)
                    si.on_wait = keep
                    changed = True
                new_insts.append(inst)
            if changed:
                bb.instructions.clear()
                for i in new_insts:
                    bb.instructions.append(i)
    return n_split


def _strip_const_loads(nc):
    """Remove preamble loads of the const page when nothing reads it."""
    from concourse import mybir

    used = set()
    removed = 0
    for f in nc.m.functions:
        for bb in f.blocks:
            for inst in bb.instructions:
                for arg in list(inst.ins):
                    t = getattr(getattr(arg, "bass_ap", None), "tensor", None)
                    n = getattr(t, "name", "") or ""
                    if n.startswith("const-"):
                        used.add(n)
    if used:
        return 0
    for f in nc.m.functions:
        for bb in f.blocks:
            keep = []
            for inst in bb.instructions:
                if type(inst).__name__ == "InstTensorLoad":
                    outs = list(inst.outs)
                    names = []
                    for a in outs:
                        t = getattr(getattr(a, "bass_ap", None), "tensor", None)
                        names.append(getattr(t, "name", "") or "")
                    if names and all(n.startswith("const-") for n in names):
                        removed += 1
                        continue
                keep.append(inst)
            if len(keep) != len(bb.instructions):
                bb.instructions.clear()
                for i in keep:
                    bb.instructions.append(i)
    return removed


def _strip_tail(nc):
    """Remove the end-of-kernel barrier/sem-reset tail.

    The runtime clears semaphores in its own exec preamble, and every
    engine's results flow into the output DMA via data-dependency
    semaphores, so the only thing that must remain is the sync-engine
    DRAIN that flushes the output DMA queue."""
    from concourse import mybir

    f = nc.m.functions[0]
    bb = f.blocks[-1]
    insts = list(bb.instructions)
    idx = None
    for i, inst in enumerate(insts):
        if isinstance(inst, mybir.InstDrain) and inst.engine == mybir.EngineType.SP:
            idx = i
            break
    if idx is None:
        return 0
    kept = insts[: idx + 1]
    drain = kept[-1]
    if drain.sync_info is not None:
        drain.sync_info.on_wait = []
    removed = len(insts) - len(kept)
    bb.instructions.clear()
    for i in kept:
        bb.instructions.append(i)
    return removed


def _np_sto_dtype(mode):
    if mode == "bf16":
        import ml_dtypes

        return ml_dtypes.bfloat16
    return np.float32


def _route(command):
    """Group sample indices by expert, pad each group to a multiple of
    8*128 and split evenly across cores.

    Returns caps [E] (rows per core per expert) and I [NCORES, R] row
    index arrays (R = sum(caps))."""
    caps = []
    parts = []  # per expert: [NCORES, cap_e] padded index array
    for e in range(NUM_COMMANDS):
        idx = np.nonzero(command == e)[0].astype(np.int64)
        n = len(idx)
        cap = int(np.ceil(n / (NCORES * P))) * P if n else 0
        caps.append(cap)
        if cap == 0:
            parts.append(np.zeros((NCORES, 0), np.int64))
            continue
        pad = NCORES * cap - n
        idx_pad = np.concatenate([idx, np.full(pad, idx[-1], np.int64)])
        parts.append(idx_pad.reshape(NCORES, cap))
    desc = sorted(range(NUM_COMMANDS), key=lambda e: -caps[e])
    # small expert first (fast DMA start), small expert last (short tail)
    order = [desc[2], desc[0], desc[1], desc[3]]
    I = [np.concatenate([parts[e][k] for e in order]) for k in range(NCORES)]
    return [caps[e] for e in order], order, np.stack(I)


def _build_program(R, caps, eorder, b2c, n_pos, mode):
    from contextlib import ExitStack

    import concourse.bass as bass
    import concourse.tile as tile
    from concourse import mybir

    f32 = mybir.dt.float32
    # matmul-operand dtype (the whole producer chain must carry it for
    # the fp32r BIR verifier) and elementwise/storage dtype
    if mode == "bf16":
        MMD = mybir.dt.bfloat16
        STO = mybir.dt.bfloat16
    elif mode == "f32r":
        MMD = mybir.dt.float32r
        STO = f32
    else:
        MMD = f32
        STO = f32
    T = R // P

    pack = os.environ.get("KERNEL_PACK_MEAS", "1") == "1"
    nc = bass.Bass()
    # all arrays are PRE-TILED on the host so every DMA is a dense
    # [partition, contiguous-bytes] copy (cheap descriptor generation)
    imgT_d = nc.declare_dram_parameter("img_pre", [P, 4 * R], MMD, isOutput=False)
    measT_d = nc.declare_dram_parameter(
        "measAug", [NUM_MEAS + 1, R], MMD, isOutput=False
    )
    A_d = nc.declare_dram_parameter("A_pre", [NUM_COMMANDS, P, 4 * EMB], MMD, isOutput=False)
    WfAug_d = nc.declare_dram_parameter(
        "WfAug_pre", [NUM_MEAS + 1, NUM_COMMANDS, EMB], MMD, isOutput=False
    )
    b2tail_d = nc.declare_dram_parameter("b2tail", [P, T], f32, isOutput=False)
    outp_d = nc.declare_dram_parameter("outp", [P, 2, T], f32, isOutput=True)

    with tile.TileContext(nc) as tc:
        with ExitStack() as ctx:
            const_pool = ctx.enter_context(tc.tile_pool(name="const", bufs=1))
            w_pool = ctx.enter_context(tc.tile_pool(name="w", bufs=16))
            img_pool = ctx.enter_context(tc.tile_pool(name="img", bufs=16))
            junk_pool = ctx.enter_context(tc.tile_pool(name="junk", bufs=4))
            out_pool = ctx.enter_context(tc.tile_pool(name="out", bufs=1))
            ps_pool = ctx.enter_context(tc.tile_pool(name="ps", bufs=6, space="PSUM"))
            psw_pool = ctx.enter_context(tc.tile_pool(name="psw", bufs=1, space="PSUM"))

            # greedy least-loaded DMA queue assignment over the three
            # DMA-capable engines (SP + ACT hwdge, Pool swdge), with
            # transfers grouped into waves chained by semaphores so the
            # first-needed expert's data doesn't share DMA bandwidth
            # with later experts' transfers
            from concourse.tile_rust import add_dep_helper

            dma_engines = [nc.sync, nc.scalar, nc.gpsimd]
            # measured queue service rates differ: sync-HW ~1.4x scalar-HW,
            # gpsimd-SW slightly slower; balance by completion time
            dma_speed = [1.4, 1.0, 0.92]
            dma_load = [0.0, 0.0, 0.0]
            waves = [[]]

            def dma(dst, src, nbytes):
                qi = dma_load.index(min(dma_load))
                dma_load[qi] += nbytes / dma_speed[qi]
                inst = dma_engines[qi].dma_start(dst, src)
                waves[-1].append(inst)
                return inst

            def next_wave():
                if waves[-1]:
                    waves.append([])

            esz = 2 if mode == "bf16" else 4
            mrows = P if pack else NUM_MEAS + 1
            nrep = 4 if pack else 1
            measT_sb = const_pool.tile([mrows, R], MMD)
            WfAug_sb = const_pool.tile([mrows, NUM_COMMANDS, EMB], MMD)
            for j in range(nrep):
                dma(
                    measT_sb[32 * j : 32 * j + NUM_MEAS + 1, :],
                    measT_d[:],
                    9 * R * esz,
                )
                dma(
                    WfAug_sb[32 * j : 32 * j + NUM_MEAS + 1, :, :],
                    WfAug_d[:],
                    9 * 4 * EMB * esz,
                )
            b2tail_sb = const_pool.tile([P, T], f32, tag="b2tail", name="b2tail_sb")
            dma(b2tail_sb[:], b2tail_d[:], P * T * 4)
            zbias = const_pool.tile([P, 1], f32)
            nc.vector.memset(zbias[:], 0.0)
            p_pos = {}
            p_neg = {}
            for i, cap in enumerate(caps):
                if cap == 0:
                    continue
                tseg = cap // P
                p_pos[i] = out_pool.tile([P, tseg], f32, tag=f"pp_{i}", name=f"pp_{i}")
                p_neg[i] = out_pool.tile([P, tseg], f32, tag=f"pn_{i}", name=f"pn_{i}")
                nc.vector.memset(p_pos[i][:], 0.0)
                nc.vector.memset(p_neg[i][:], 0.0)

            A_sb = {}
            img_sb = {}
            for i, cap in enumerate(caps):
                if cap == 0:
                    continue
                next_wave()
                e = eorder[i]
                base = 4 * sum(caps[:i])
                for c in range(4):
                    A_sb[i, c] = w_pool.tile(
                        [P, EMB], MMD, tag="A", name=f"A_sb_{i}_{c}"
                    )
                    dma(
                        A_sb[i, c][:],
                        A_d[e][:, c * EMB : (c + 1) * EMB],
                        P * EMB * esz,
                    )
                    img_sb[i, c] = img_pool.tile(
                        [P, cap], MMD, tag="img", name=f"img_sb_{i}_{c}"
                    )
                    dma(
                        img_sb[i, c][:],
                        imgT_d[:, base + c * cap : base + (c + 1) * cap],
                        P * cap * esz,
                    )

            if os.environ.get("KERNEL_WAVES", "0") == "1":
                for k in range(1, len(waves)):
                    gate = waves[k - 1][-1]
                    for inst in waves[k]:
                        add_dep_helper(
                            inst.ins, gate.ins, sync=True, reason="dma wave chaining"
                        )

            # keep the PE busy during the initial DMA window so the HAM
            # clock gate is warm when real matmuls start
            warm_a = const_pool.tile([P, EMB], MMD, tag="warm_a", name="warm_a")
            nc.vector.memset(warm_a[:], 0.0)
            ps_w = psw_pool.tile([P, EMB], f32, tag="warm_ps", name="ps_warm")
            N_WARM = 16
            for w in range(N_WARM):
                nc.tensor.matmul(
                    ps_w[:],
                    lhsT=warm_a[:, :P],
                    rhs=warm_a[:],
                    start=(w == 0),
                    stop=(w == N_WARM - 1),
                )
            junkw = junk_pool.tile([P, EMB], STO, tag="junk")
            nc.scalar.activation(
                junkw[:], ps_w[:], mybir.ActivationFunctionType.Copy
            )

            for i, cap in enumerate(caps):
                e = eorder[i]
                off = sum(caps[:i])
                nt = cap // P
                group, ps_of = {}, {}
                for r0 in range(0, nt, 4):
                    group[r0] = list(range(r0, min(r0 + 4, nt)))
                for r in range(nt):
                    if pack and r in group:
                        # emit the whole group's img matmuls, then the
                        # packed meas matmuls as concurrent row-groups
                        for rr in group[r]:
                            psr = ps_pool.tile(
                                [P, EMB], f32, tag="h", name=f"ps_{i}_{rr}"
                            )
                            ps_of[rr] = psr
                            for ko in range(4):
                                nc.tensor.matmul(
                                    psr[:],
                                    lhsT=img_sb[i, ko][:, rr * P : (rr + 1) * P],
                                    rhs=A_sb[i, ko][:],
                                    start=(ko == 0),
                                    stop=False,
                                )
                        for j, rr in enumerate(group[r]):
                            col = off + rr * P
                            nc.tensor.matmul(
                                ps_of[rr][:],
                                lhsT=measT_sb[
                                    32 * j : 32 * j + NUM_MEAS + 1, col : col + P
                                ],
                                rhs=WfAug_sb[32 * j : 32 * j + NUM_MEAS + 1, e, :],
                                start=False,
                                stop=True,
                                tile_position=(32 * j, 0),
                            )
                    if pack:
                        ps = ps_of[r]
                    else:
                        ps = ps_pool.tile([P, EMB], f32, tag="h")
                        for ko in range(4):
                            nc.tensor.matmul(
                                ps[:],
                                lhsT=img_sb[i, ko][:, r * P : (r + 1) * P],
                                rhs=A_sb[i, ko][:],
                                start=(ko == 0),
                                stop=False,
                            )
                        col = off + r * P
                        nc.tensor.matmul(
                            ps[:],
                            lhsT=measT_sb[:, col : col + P],
                            rhs=WfAug_sb[:, e, :],
                            start=False,
                            stop=True,
                        )
                    junk = junk_pool.tile([P, EMB], STO, tag="junk")
                    npe = n_pos[e]
                    if npe > 0:
                        nc.scalar.activation(
                            junk[:, :npe],
                            ps[:, :npe],
                            mybir.ActivationFunctionType.Relu,
                            bias=zbias[:],
                            accum_out=p_pos[i][:, r : r + 1],
                        )
                    if npe < EMB:
                        junk2 = junk_pool.tile([P, EMB], STO, tag="junk2")
                        nc.vector.tensor_scalar(
                            junk2[:, npe:],
                            ps[:, npe:],
                            0.0,
                            0.0,
                            mybir.AluOpType.max,
                            mybir.AluOpType.add,
                            accum_out=p_neg[i][:, r : r + 1],
                        )

                tseg = cap // P
                seg = slice(off // P, off // P + tseg)
                q = out_pool.tile([P, tseg], f32, tag=f"q_{i}", name=f"q_{i}")
                sig = out_pool.tile([P, tseg], f32, tag=f"sig_{i}", name=f"sig_{i}")
                outs = out_pool.tile(
                    [P, 2, tseg], f32, tag=f"outs_{i}", name=f"outs_{i}"
                )
                nc.vector.tensor_tensor(
                    q[:], p_pos[i][:], p_neg[i][:], mybir.AluOpType.subtract
                )
                nc.vector.tensor_add(q[:], q[:], b2tail_sb[:, seg])
                nc.scalar.activation(
                    sig[:],
                    q[:],
                    mybir.ActivationFunctionType.Sigmoid,
                    bias=zbias[:],
                )
                nc.vector.tensor_scalar_mul(outs[:, 0, :], sig[:], 50.0)
                nc.vector.tensor_scalar(
                    outs[:, 1, :],
                    q[:],
                    1.0,
                    -1.0,
                    mybir.AluOpType.min,
                    mybir.AluOpType.max,
                )
                dma(outp_d[:, :, seg], outs[:], P * 2 * tseg * 4)



    _strip_tail(nc)
    _split_excess_waits(nc)
    return nc


def _prepare(inputs, mode):
    img_embs = np.asarray(inputs["img_embs"], np.float32)
    measurements = np.asarray(inputs["measurements"], np.float32)
    command = np.asarray(inputs["command"])
    W_meas = np.asarray(inputs["W_meas"], np.float32)
    b_meas = np.asarray(inputs["b_meas"], np.float32)
    W1 = np.asarray(inputs["W1"], np.float32)
    b1 = np.asarray(inputs["b1"], np.float32)
    W2 = np.asarray(inputs["W2"], np.float32)
    b2 = np.asarray(inputs["b2"], np.float32)

    sto = _np_sto_dtype(mode)
    caps, eorder, I = _route(command)
    R = int(sum(caps))

    # fold measurement path (float64 for the host-side precompute)
    W1h = W1[:, EMB:, :].astype(np.float64)
    Wf = np.einsum("md,edh->emh", W_meas.astype(np.float64), W1h)
    b_eff = np.einsum("d,edh->eh", b_meas.astype(np.float64), W1h) + b1
    A64 = W1[:, :EMB, :].astype(np.float64)

    # fold |w2[:, 0]| into the hidden columns and permute them so the
    # w2>0 columns come first: p = sum(relu(pos cols)) - sum(relu(neg
    # cols)), computed for free by the ACT accum during the relu pass.
    w2c = W2[:, :, 0].astype(np.float64)
    n_pos = []
    A_s = np.empty_like(A64)
    Wf_s = np.empty_like(Wf)
    b_eff_s = np.empty_like(b_eff)
    for e in range(NUM_COMMANDS):
        perm = np.argsort(w2c[e] <= 0, kind="stable")
        n_pos.append(int((w2c[e] > 0).sum()))
        sc = np.abs(w2c[e])[perm]
        A_s[e] = A64[e][:, perm] * sc[None, :]
        Wf_s[e] = Wf[e][:, perm] * sc[None, :]
        b_eff_s[e] = b_eff[e][perm] * sc
    WfAug = np.concatenate([Wf_s, b_eff_s[:, None, :]], axis=1).astype(sto)
    A = np.ascontiguousarray(A_s).astype(sto)  # [E,512,512]
    b2c = [float(x) for x in b2[:, 0]]

    T = R // P
    col_expert = np.concatenate(
        [np.full(caps[i] // P, eorder[i], np.int64) for i in range(NUM_COMMANDS)]
    )
    b2tail = np.broadcast_to(
        np.array([b2c[e] for e in col_expert], np.float32)[None, :], (P, T)
    ).copy()

    # pre-tiled shared weights: every device DMA is a dense 2D copy
    A_pre = np.ascontiguousarray(
        A.reshape(NUM_COMMANDS, 4, P, EMB).transpose(0, 2, 1, 3).reshape(
            NUM_COMMANDS, P, 4 * EMB
        )
    )
    WfAug_pre = np.ascontiguousarray(WfAug.transpose(1, 0, 2))  # [9, E, 512]

    imgT = img_embs.T.astype(sto)  # [512, B] cast once
    measT = measurements.T  # [8, B]
    ones_row = np.ones((1, R), np.float32).astype(sto)
    in_maps = []
    for k in range(NCORES):
        Ik = I[k]
        imgT_k = imgT[:, Ik].reshape(4, P, R)  # [o, p, r]
        img_pre = np.concatenate(
            [
                imgT_k[:, :, sum(caps[:e]) : sum(caps[: e + 1])]
                .transpose(1, 0, 2)
                .reshape(P, 4 * caps[e])
                for e in range(NUM_COMMANDS)
                if caps[e]
            ],
            axis=1,
        )
        measAug_k = np.concatenate(
            [measT[:, Ik].astype(sto), ones_row], axis=0
        )
        in_maps.append(
            {
                "img_pre": np.ascontiguousarray(img_pre),
                "measAug": measAug_k,
                "A_pre": A_pre,
                "WfAug_pre": WfAug_pre,
                "b2tail": b2tail,
            }
        )
    return in_maps, I, R, caps, eorder, b2c, n_pos


def _run(inputs, mode=None, trace=False):
    """Returns ((angle, speed), BassKernelResults)."""
    mode = mode or MODE
    _install_ntff_shim()
    from concourse.bass_utils import run_bass_kernel_spmd

    in_maps, I, R, caps, eorder, b2c, n_pos = _prepare(inputs, mode)
    key = (
        R,
        tuple(caps),
        tuple(eorder),
        mode,
        tuple(np.float32(b) for b in b2c),
        tuple(n_pos),
    )
    if key not in _CACHE:
        _CACHE[key] = _build_program(R, caps, eorder, b2c, n_pos, mode)
    nc = _CACHE[key]

    res = run_bass_kernel_spmd(
        nc, in_maps, core_ids=list(range(NCORES)), trace=trace
    )

    angle = np.zeros(B, np.float32)
    speed = np.zeros(B, np.float32)
    for k in range(NCORES):
        outp = res.results[k]["outp"]  # [128, 2, T]
        Ik = I[k]
        angle[Ik] = outp[:, 0, :].T.reshape(R)
        speed[Ik] = outp[:, 1, :].T.reshape(R)
    return (angle, speed), res


def kernel(**inputs):
    out, _ = _run(inputs)
    return out


# revision 23
# speedup vs baseline: 1.1073x; 1.0672x over previous
"""Trainium2 Bass kernel for nn_BranchedNetwork (moe_routing).

Computation (reference):
    meas_embs = measurements @ W_meas + b_meas           [B, 512]
    embs      = concat([img_embs, meas_embs], axis=1)    [B, 1024]
    h_e       = relu(embs @ W1[e] + b1[e])               per expert e
    out_e     = h_e @ W2[e] + b2[e]
    p[i]      = out[command[i], i, 0]
    angle     = sigmoid(p) * 50 ; speed = clip(p, -1, 1)

Strategy:
  * Per-sample routing is done on the host: samples are grouped by
    command id, each group padded to a multiple of 8*128 rows and
    split evenly over the 8 cores (data parallel, weights replicated).
  * Only the selected expert runs per sample (4x less compute), and
    only column 0 of W2 is needed.
  * The measurement path is folded on the host:
      h_pre = img @ W1[e][:512] + meas @ (W_meas @ W1[e][512:])
              + (b_meas @ W1[e][512:] + b1[e])
    so the device contraction is K = 512 (img) + 8 (meas) + 1 (bias
    via a ones row) instead of 1024.
  * Device per 128-row tile: 5 PE matmuls (psum [128 rows, 512 hid]),
    ACT relu psum->sbuf, then one DVE tensor_tensor_reduce computing
    p = relu_h @ w2col + b2 as a fused multiply + free-dim reduction.
    Final sigmoid/scale/clip on [128, n_tiles] at full lane width.
"""

import os
import sys
import types

import numpy as np

if "/opt/trn_rl_repo" not in sys.path and not any(
    p.endswith("trn_rl_repo") for p in sys.path
):
    sys.path.insert(0, "/opt/trn_rl_repo")

B = 16384
EMB = 512
NUM_COMMANDS = 4
NUM_MEAS = 8
NCORES = 8
P = 128

# matmul dtype mode: "f32" (exact, 4 cyc/row), "f32r" (full speed,
# reduced internal precision), "bf16" (full speed + half DMA traffic)
MODE = os.environ.get("KERNEL_MM_MODE", "f32r")

_CACHE = {}


def _install_ntff_shim():
    """Recreate antenv.axon_hooks so trace=True works if requested."""
    if "antenv.axon_hooks" in sys.modules:
        return
    try:
        import antenv

        mod = types.ModuleType("antenv.axon_hooks")
        mod._hook = None
        mod.set_axon_ntff_profile_hook = lambda h: setattr(mod, "_hook", h)
        mod.get_axon_ntff_profile_hook = lambda: mod._hook
        sys.modules["antenv.axon_hooks"] = mod
        antenv.axon_hooks = mod
        from trn_agent_boot.trn_boot import _ntff_profile_via_ctypes

        mod.set_axon_ntff_profile_hook(
            _ntff_profile_via_ctypes("/opt/axon/libaxon_pjrt.so")
        )
    except Exception:
        pass


def _split_excess_waits(nc, max_waits=1):
    """The walrus in this container rejects instructions with more than
    one embedded sync-wait command. Waits execute in order on the
    issuing engine, so hoisting the excess onto preceding NOPs on the
    same engine is semantically identical."""
    from concourse import mybir

    n_split = 0
    for f in nc.m.functions:
        for bb in f.blocks:
            insts = list(bb.instructions)
            new_insts = []
            changed = False
            for inst in insts:
                si = inst.sync_info
                if si is not None and si.on_wait and len(si.on_wait) > max_waits:
                    waits = list(si.on_wait)
                    extra, keep = waits[:-max_waits], waits[-max_waits:]
                    while extra:
                        chunk, extra = extra[:max_waits], extra[max_waits:]
                        n_split += 1
                        nop = mybir.InstNoOp(
                            name=f"waitsplit_{n_split}_{inst.name}",
                            engine=inst.engine,
                            ins=[],
                            outs=[],
                            sync_info=mybir.SyncInfo(on_wait=chunk, on_update=[]),
                        )
                        new_insts.append(nop)
                    si.on_wait = keep
                    changed = True
                new_insts.append(inst)
            if changed:
                bb.instructions.clear()
                for i in new_insts:
                    bb.instructions.append(i)
    return n_split


def _strip_const_loads(nc):
    """Remove preamble loads of the const page when nothing reads it."""
    from concourse import mybir

    used = set()
    removed = 0
    for f in nc.m.functions:
        for bb in f.blocks:
            for inst in bb.instructions:
                for arg in list(inst.ins):
                    t = getattr(getattr(arg, "bass_ap", None), "tensor", None)
                    n = getattr(t, "name", "") or ""
                    if n.startswith("const-"):
                        used.add(n)
    if used:
        return 0
    for f in nc.m.functions:
        for bb in f.blocks:
            keep = []
            for inst in bb.instructions:
                if type(inst).__name__ == "InstTensorLoad":
                    outs = list(inst.outs)
                    names = []
                    for a in outs:
                        t = getattr(getattr(a, "bass_ap", None), "tensor", None)
                        names.append(getattr(t, "name", "") or "")
                    if names and all(n.startswith("const-") for n in names):
                        removed += 1
                        continue
                keep.append(inst)
            if len(keep) != len(bb.instructions):
                bb.instructions.clear()
                for i in keep:
                    bb.instructions.append(i)
    return removed


def _strip_tail(nc):
    """Remove the end-of-kernel barrier/sem-reset tail.

    The runtime clears semaphores in its own exec preamble, and every
    engine's results flow into the output DMA via data-dependency
    semaphores, so the only thing that must remain is the sync-engine
    DRAIN that flushes the output DMA queue."""
    from concourse import mybir

    f = nc.m.functions[0]
    bb = f.blocks[-1]
    insts = list(bb.instructions)
    idx = None
    for i, inst in enumerate(insts):
        if isinstance(inst, mybir.InstDrain) and inst.engine == mybir.EngineType.SP:
            idx = i
            break
    if idx is None:
        return 0
    kept = insts[: idx + 1]
    drain = kept[-1]
    if drain.sync_info is not None:
        drain.sync_info.on_wait = []
    removed = len(insts) - len(kept)
    bb.instructions.clear()
    for i in kept:
        bb.instructions.append(i)
    return removed


def _np_sto_dtype(mode):
    if mode == "bf16":
        import ml_dtypes

        return ml_dtypes.bfloat16
    return np.float32


def _route(command):
    """Group sample indices by expert, pad each group to a multiple of
    8*128 and split evenly across cores.

    Returns caps [E] (rows per core per expert) and I [NCORES, R] row
    index arrays (R = sum(caps))."""
    caps = []
    parts = []  # per expert: [NCORES, cap_e] padded index array
    for e in range(NUM_COMMANDS):
        idx = np.nonzero(command == e)[0].astype(np.int64)
        n = len(idx)
        cap = int(np.ceil(n / (NCORES * P))) * P if n else 0
        caps.append(cap)
        if cap == 0:
            parts.append(np.zeros((NCORES, 0), np.int64))
            continue
        pad = NCORES * cap - n
        idx_pad = np.concatenate([idx, np.full(pad, idx[-1], np.int64)])
        parts.append(idx_pad.reshape(NCORES, cap))
    desc = sorted(range(NUM_COMMANDS), key=lambda e: -caps[e])
    # small expert first (fast DMA start), small expert last (short tail)
    order = [desc[2], desc[0], desc[1], desc[3]]
    I = [np.concatenate([parts[e][k] for e in order]) for k in range(NCORES)]
    return [caps[e] for e in order], order, np.stack(I)


def _build_program(R, caps, eorder, b2c, n_pos, mode):
    from contextlib import ExitStack

    import concourse.bass as bass
    import concourse.tile as tile
    from concourse import mybir

    f32 = mybir.dt.float32
    # matmul-operand dtype (the whole producer chain must carry it for
    # the fp32r BIR verifier) and elementwise/storage dtype
    if mode == "bf16":
        MMD = mybir.dt.bfloat16
        STO = mybir.dt.bfloat16
    elif mode == "f32r":
        MMD = mybir.dt.float32r
        STO = f32
    else:
        MMD = f32
        STO = f32
    T = R // P

    pack = os.environ.get("KERNEL_PACK_MEAS", "1") == "1"
    nc = bass.Bass()
    # all arrays are PRE-TILED on the host so every DMA is a dense
    # [partition, contiguous-bytes] copy (cheap descriptor generation)
    imgT_d = nc.declare_dram_parameter("img_pre", [P, 4 * R], MMD, isOutput=False)
    measT_d = nc.declare_dram_parameter(
        "measAug", [NUM_MEAS + 1, R], MMD, isOutput=False
    )
    A_d = nc.declare_dram_parameter("A_pre", [NUM_COMMANDS, P, 4 * EMB], MMD, isOutput=False)
    WfAug_d = nc.declare_dram_parameter(
        "WfAug_pre", [NUM_MEAS + 1, NUM_COMMANDS, EMB], MMD, isOutput=False
    )
    b2tail_d = nc.declare_dram_parameter("b2tail", [P, T], f32, isOutput=False)
    outp_d = nc.declare_dram_parameter("outp", [P, 2, T], f32, isOutput=True)

    with tile.TileContext(nc) as tc:
        with ExitStack() as ctx:
            const_pool = ctx.enter_context(tc.tile_pool(name="const", bufs=1))
            w_pool = ctx.enter_context(tc.tile_pool(name="w", bufs=16))
            img_pool = ctx.enter_context(tc.tile_pool(name="img", bufs=16))
            junk_pool = ctx.enter_context(tc.tile_pool(name="junk", bufs=4))
            out_pool = ctx.enter_context(tc.tile_pool(name="out", bufs=1))
            ps_pool = ctx.enter_context(tc.tile_pool(name="ps", bufs=6, space="PSUM"))
            psw_pool = ctx.enter_context(tc.tile_pool(name="psw", bufs=1, space="PSUM"))

            # greedy least-loaded DMA queue assignment over the three
            # DMA-capable engines (SP + ACT hwdge, Pool swdge), with
            # transfers grouped into waves chained by semaphores so the
            # first-needed expert's data doesn't share DMA bandwidth
            # with later experts' transfers
            from concourse.tile_rust import add_dep_helper

            dma_engines = [nc.sync, nc.scalar, nc.gpsimd]
            # measured queue service rates differ: sync-HW ~1.4x scalar-HW,
            # gpsimd-SW slightly slower; balance by completion time
            dma_speed = [1.4, 1.0, 0.92]
            dma_load = [0.0, 0.0, 0.0]
            waves = [[]]

            def dma(dst, src, nbytes):
                qi = dma_load.index(min(dma_load))
                dma_load[qi] += nbytes / dma_speed[qi]
                inst = dma_engines[qi].dma_start(dst, src)
                waves[-1].append(inst)
                return inst

            def next_wave():
                if waves[-1]:
                    waves.append([])

            esz = 2 if mode == "bf16" else 4
            mrows = P if pack else NUM_MEAS + 1
            nrep = 4 if pack else 1
            measT_sb = const_pool.tile([mrows, R], MMD)
            WfAug_sb = const_pool.tile([mrows, NUM_COMMANDS, EMB], MMD)
            for j in range(nrep):
                dma(
                    measT_sb[32 * j : 32 * j + NUM_MEAS + 1, :],
                    measT_d[:],
                    9 * R * esz,
                )
                dma(
                    WfAug_sb[32 * j : 32 * j + NUM_MEAS + 1, :, :],
                    WfAug_d[:],
                    9 * 4 * EMB * esz,
                )
            b2tail_sb = const_pool.tile([P, T], f32, tag="b2tail", name="b2tail_sb")
            dma(b2tail_sb[:], b2tail_d[:], P * T * 4)
            zbias = const_pool.tile([P, 1], f32)
            nc.vector.memset(zbias[:], 0.0)
            p_pos = {}
            p_neg = {}
            for i, cap in enumerate(caps):
                if cap == 0:
                    continue
                tseg = cap // P
                p_pos[i] = out_pool.tile([P, tseg], f32, tag=f"pp_{i}", name=f"pp_{i}")
                p_neg[i] = out_pool.tile([P, tseg], f32, tag=f"pn_{i}", name=f"pn_{i}")
                nc.vector.memset(p_pos[i][:], 0.0)
                nc.vector.memset(p_neg[i][:], 0.0)

            A_sb = {}
            img_sb = {}
            for i, cap in enumerate(caps):
                if cap == 0:
                    continue
                next_wave()
                e = eorder[i]
                base = 4 * sum(caps[:i])
                for c in range(4):
                    A_sb[i, c] = w_pool.tile(
                        [P, EMB], MMD, tag="A", name=f"A_sb_{i}_{c}"
                    )
                    dma(
                        A_sb[i, c][:],
                        A_d[e][:, c * EMB : (c + 1) * EMB],
                        P * EMB * esz,
                    )
                    img_sb[i, c] = img_pool.tile(
                        [P, cap], MMD, tag="img", name=f"img_sb_{i}_{c}"
                    )
                    dma(
                        img_sb[i, c][:],
                        imgT_d[:, base + c * cap : base + (c + 1) * cap],
                        P * cap * esz,
                    )

            if os.environ.get("KERNEL_WAVES", "0") == "1":
                for k in range(1, len(waves)):
                    gate = waves[k - 1][-1]
                    seen_eng = set()
                    for inst in waves[k]:
                        eng = inst.ins.engine
                        if eng in seen_eng:
                            continue
                        seen_eng.add(eng)
                        add_dep_helper(
                            inst.ins, gate.ins, sync=True, reason="dma wave chaining"
                        )

            # keep the PE busy during the initial DMA window so the HAM
            # clock gate is warm when real matmuls start
            warm_a = const_pool.tile([P, EMB], MMD, tag="warm_a", name="warm_a")
            nc.vector.memset(warm_a[:], 0.0)
            ps_w = psw_pool.tile([P, EMB], f32, tag="warm_ps", name="ps_warm")
            N_WARM = 16
            for w in range(N_WARM):
                nc.tensor.matmul(
                    ps_w[:],
                    lhsT=warm_a[:, :P],
                    rhs=warm_a[:],
                    start=(w == 0),
                    stop=(w == N_WARM - 1),
                )
            junkw = junk_pool.tile([P, EMB], STO, tag="junk")
            nc.scalar.activation(
                junkw[:], ps_w[:], mybir.ActivationFunctionType.Copy
            )

            for i, cap in enumerate(caps):
                e = eorder[i]
                off = sum(caps[:i])
                nt = cap // P
                group, ps_of = {}, {}
                for r0 in range(0, nt, 4):
                    group[r0] = list(range(r0, min(r0 + 4, nt)))
                for r in range(nt):
                    if pack and r in group:
                        # packed meas matmuls FIRST (start=True, concurrent
                        # row-groups), then each tile's img matmuls; the
                        # per-tile accum follows its own last img matmul
                        for j, rr in enumerate(group[r]):
                            psr = ps_pool.tile(
                                [P, EMB], f32, tag="h", name=f"ps_{i}_{rr}"
                            )
                            ps_of[rr] = psr
                            col = off + rr * P
                            nc.tensor.matmul(
                                psr[:],
                                lhsT=measT_sb[
                                    32 * j : 32 * j + NUM_MEAS + 1, col : col + P
                                ],
                                rhs=WfAug_sb[32 * j : 32 * j + NUM_MEAS + 1, e, :],
                                start=True,
                                stop=False,
                                tile_position=(32 * j, 0),
                            )
                        for rr in group[r]:
                            for ko in range(4):
                                nc.tensor.matmul(
                                    ps_of[rr][:],
                                    lhsT=img_sb[i, ko][:, rr * P : (rr + 1) * P],
                                    rhs=A_sb[i, ko][:],
                                    start=False,
                                    stop=(ko == 3),
                                )
                    if pack:
                        ps = ps_of[r]
                    else:
                        ps = ps_pool.tile([P, EMB], f32, tag="h")
                        for ko in range(4):
                            nc.tensor.matmul(
                                ps[:],
                                lhsT=img_sb[i, ko][:, r * P : (r + 1) * P],
                                rhs=A_sb[i, ko][:],
                                start=(ko == 0),
                                stop=False,
                            )
                        col = off + r * P
                        nc.tensor.matmul(
                            ps[:],
                            lhsT=measT_sb[:, col : col + P],
                            rhs=WfAug_sb[:, e, :],
                            start=False,
                            stop=True,
                        )
                    junk = junk_pool.tile([P, EMB], STO, tag="junk")
                    npe = n_pos[e]
                    if npe > 0:
                        nc.scalar.activation(
                            junk[:, :npe],
                            ps[:, :npe],
                            mybir.ActivationFunctionType.Relu,
                            bias=zbias[:],
                            accum_out=p_pos[i][:, r : r + 1],
                        )
                    if npe < EMB:
                        junk2 = junk_pool.tile([P, EMB], STO, tag="junk2")
                        nc.vector.tensor_scalar(
                            junk2[:, npe:],
                            ps[:, npe:],
                            0.0,
                            0.0,
                            mybir.AluOpType.max,
                            mybir.AluOpType.add,
                            accum_out=p_neg[i][:, r : r + 1],
                        )

                tseg = cap // P
                seg = slice(off // P, off // P + tseg)
                q = out_pool.tile([P, tseg], f32, tag=f"q_{i}", name=f"q_{i}")
                sig = out_pool.tile([P, tseg], f32, tag=f"sig_{i}", name=f"sig_{i}")
                outs = out_pool.tile(
                    [P, 2, tseg], f32, tag=f"outs_{i}", name=f"outs_{i}"
                )
                nc.vector.tensor_tensor(
                    q[:], p_pos[i][:], p_neg[i][:], mybir.AluOpType.subtract
                )
                nc.vector.tensor_add(q[:], q[:], b2tail_sb[:, seg])
                nc.scalar.activation(
                    sig[:],
                    q[:],
                    mybir.ActivationFunctionType.Sigmoid,
                    bias=zbias[:],
                )
                nc.vector.tensor_scalar_mul(outs[:, 0, :], sig[:], 50.0)
                nc.vector.tensor_scalar(
                    outs[:, 1, :],
                    q[:],
                    1.0,
                    -1.0,
                    mybir.AluOpType.min,
                    mybir.AluOpType.max,
                )
                dma(outp_d[:, :, seg], outs[:], P * 2 * tseg * 4)



    _strip_tail(nc)
    _split_excess_waits(nc)
    return nc


def _prepare(inputs, mode):
    img_embs = np.asarray(inputs["img_embs"], np.float32)
    measurements = np.asarray(inputs["measurements"], np.float32)
    command = np.asarray(inputs["command"])
    W_meas = np.asarray(inputs["W_meas"], np.float32)
    b_meas = np.asarray(inputs["b_meas"], np.float32)
    W1 = np.asarray(inputs["W1"], np.float32)
    b1 = np.asarray(inputs["b1"], np.float32)
    W2 = np.asarray(inputs["W2"], np.float32)
    b2 = np.asarray(inputs["b2"], np.float32)

    sto = _np_sto_dtype(mode)
    caps, eorder, I = _route(command)
    R = int(sum(caps))

    # fold measurement path (float64 for the host-side precompute)
    W1h = W1[:, EMB:, :].astype(np.float64)
    Wf = np.einsum("md,edh->emh", W_meas.astype(np.float64), W1h)
    b_eff = np.einsum("d,edh->eh", b_meas.astype(np.float64), W1h) + b1
    A64 = W1[:, :EMB, :].astype(np.float64)

    # fold |w2[:, 0]| into the hidden columns and permute them so the
    # w2>0 columns come first: p = sum(relu(pos cols)) - sum(relu(neg
    # cols)), computed for free by the ACT accum during the relu pass.
    w2c = W2[:, :, 0].astype(np.float64)
    n_pos = []
    A_s = np.empty_like(A64)
    Wf_s = np.empty_like(Wf)
    b_eff_s = np.empty_like(b_eff)
    for e in range(NUM_COMMANDS):
        perm = np.argsort(w2c[e] <= 0, kind="stable")
        n_pos.append(int((w2c[e] > 0).sum()))
        sc = np.abs(w2c[e])[perm]
        A_s[e] = A64[e][:, perm] * sc[None, :]
        Wf_s[e] = Wf[e][:, perm] * sc[None, :]
        b_eff_s[e] = b_eff[e][perm] * sc
    WfAug = np.concatenate([Wf_s, b_eff_s[:, None, :]], axis=1).astype(sto)
    A = np.ascontiguousarray(A_s).astype(sto)  # [E,512,512]
    b2c = [float(x) for x in b2[:, 0]]

    T = R // P
    col_expert = np.concatenate(
        [np.full(caps[i] // P, eorder[i], np.int64) for i in range(NUM_COMMANDS)]
    )
    b2tail = np.broadcast_to(
        np.array([b2c[e] for e in col_expert], np.float32)[None, :], (P, T)
    ).copy()

    # pre-tiled shared weights: every device DMA is a dense 2D copy
    A_pre = np.ascontiguousarray(
        A.reshape(NUM_COMMANDS, 4, P, EMB).transpose(0, 2, 1, 3).reshape(
            NUM_COMMANDS, P, 4 * EMB
        )
    )
    WfAug_pre = np.ascontiguousarray(WfAug.transpose(1, 0, 2))  # [9, E, 512]

    imgT = img_embs.T.astype(sto)  # [512, B] cast once
    measT = measurements.T  # [8, B]
    ones_row = np.ones((1, R), np.float32).astype(sto)
    in_maps = []
    for k in range(NCORES):
        Ik = I[k]
        imgT_k = imgT[:, Ik].reshape(4, P, R)  # [o, p, r]
        img_pre = np.concatenate(
            [
                imgT_k[:, :, sum(caps[:e]) : sum(caps[: e + 1])]
                .transpose(1, 0, 2)
                .reshape(P, 4 * caps[e])
                for e in range(NUM_COMMANDS)
                if caps[e]
            ],
            axis=1,
        )
        measAug_k = np.concatenate(
            [measT[:, Ik].astype(sto), ones_row], axis=0
        )
        in_maps.append(
            {
                "img_pre": np.ascontiguousarray(img_pre),
                "measAug": measAug_k,
                "A_pre": A_pre,
                "WfAug_pre": WfAug_pre,
                "b2tail": b2tail,
            }
        )
    return in_maps, I, R, caps, eorder, b2c, n_pos


def _run(inputs, mode=None, trace=False):
    """Returns ((angle, speed), BassKernelResults)."""
    mode = mode or MODE
    _install_ntff_shim()
    from concourse.bass_utils import run_bass_kernel_spmd

    in_maps, I, R, caps, eorder, b2c, n_pos = _prepare(inputs, mode)
    key = (
        R,
        tuple(caps),
        tuple(eorder),
        mode,
        tuple(np.float32(b) for b in b2c),
        tuple(n_pos),
    )
    if key not in _CACHE:
        _CACHE[key] = _build_program(R, caps, eorder, b2c, n_pos, mode)
    nc = _CACHE[key]

    res = run_bass_kernel_spmd(
        nc, in_maps, core_ids=list(range(NCORES)), trace=trace
    )

    angle = np.zeros(B, np.float32)
    speed = np.zeros(B, np.float32)
    for k in range(NCORES):
        outp = res.results[k]["outp"]  # [128, 2, T]
        Ik = I[k]
        angle[Ik] = outp[:, 0, :].T.reshape(R)
        speed[Ik] = outp[:, 1, :].T.reshape(R)
    return (angle, speed), res


def kernel(**inputs):
    out, _ = _run(inputs)
    return out


# revision 24
# speedup vs baseline: 1.1200x; 1.0115x over previous
"""Trainium2 Bass kernel for nn_BranchedNetwork (moe_routing).

Computation (reference):
    meas_embs = measurements @ W_meas + b_meas           [B, 512]
    embs      = concat([img_embs, meas_embs], axis=1)    [B, 1024]
    h_e       = relu(embs @ W1[e] + b1[e])               per expert e
    out_e     = h_e @ W2[e] + b2[e]
    p[i]      = out[command[i], i, 0]
    angle     = sigmoid(p) * 50 ; speed = clip(p, -1, 1)

Strategy:
  * Per-sample routing is done on the host: samples are grouped by
    command id, each group padded to a multiple of 8*128 rows and
    split evenly over the 8 cores (data parallel, weights replicated).
  * Only the selected expert runs per sample (4x less compute), and
    only column 0 of W2 is needed.
  * The measurement path is folded on the host:
      h_pre = img @ W1[e][:512] + meas @ (W_meas @ W1[e][512:])
              + (b_meas @ W1[e][512:] + b1[e])
    so the device contraction is K = 512 (img) + 8 (meas) + 1 (bias
    via a ones row) instead of 1024.
  * |w2[:, 0]| is folded into the layer-1 weights with hidden columns
    permuted by sign of w2, so layer 2 reduces to
    p = sum(relu(pos cols)) - sum(relu(neg cols)) + b2, computed for
    free by ACT/DVE accumulators during the relu pass.
  * Device per 128-row tile: a packed K=9 meas matmul (4 tiles run
    concurrently in separate PE row-groups via tile_position) + 4
    K=128 img matmuls accumulate psum [128 rows, 512 hid]; ACT does
    relu+accum on the positive columns, DVE on the negative ones.
  * bf16 operands (fp32 accumulation in PSUM), host-pre-tiled layouts
    so every DMA is a dense 2D copy, DMAs load-balanced over the
    sync/scalar/gpsimd queues, PE warmed up with dummy matmuls during
    the initial DMA window, and the framework's end-of-kernel
    barrier/sem-reset tail stripped.
"""

import os
import sys
import types

import numpy as np

if "/opt/trn_rl_repo" not in sys.path and not any(
    p.endswith("trn_rl_repo") for p in sys.path
):
    sys.path.insert(0, "/opt/trn_rl_repo")

B = 16384
EMB = 512
NUM_COMMANDS = 4
NUM_MEAS = 8
NCORES = 8
P = 128

# matmul dtype mode: "f32" (exact, 4 cyc/row), "f32r" (full speed,
# reduced internal precision), "bf16" (full speed + half DMA traffic)
MODE = os.environ.get("KERNEL_MM_MODE", "bf16")

_CACHE = {}


def _install_ntff_shim():
    """Recreate antenv.axon_hooks so trace=True works if requested."""
    if "antenv.axon_hooks" in sys.modules:
        return
    try:
        import antenv

        mod = types.ModuleType("antenv.axon_hooks")
        mod._hook = None
        mod.set_axon_ntff_profile_hook = lambda h: setattr(mod, "_hook", h)
        mod.get_axon_ntff_profile_hook = lambda: mod._hook
        sys.modules["antenv.axon_hooks"] = mod
        antenv.axon_hooks = mod
        from trn_agent_boot.trn_boot import _ntff_profile_via_ctypes

        mod.set_axon_ntff_profile_hook(
            _ntff_profile_via_ctypes("/opt/axon/libaxon_pjrt.so")
        )
    except Exception:
        pass


def _split_excess_waits(nc, max_waits=1):
    """The walrus in this container rejects instructions with more than
    one embedded sync-wait command. Waits execute in order on the
    issuing engine, so hoisting the excess onto preceding NOPs on the
    same engine is semantically identical."""
    from concourse import mybir

    n_split = 0
    for f in nc.m.functions:
        for bb in f.blocks:
            insts = list(bb.instructions)
            new_insts = []
            changed = False
            for inst in insts:
                si = inst.sync_info
                if si is not None and si.on_wait and len(si.on_wait) > max_waits:
                    waits = list(si.on_wait)
                    extra, keep = waits[:-max_waits], waits[-max_waits:]
                    while extra:
                        chunk, extra = extra[:max_waits], extra[max_waits:]
                        n_split += 1
                        nop = mybir.InstNoOp(
                            name=f"waitsplit_{n_split}_{inst.name}",
                            engine=inst.engine,
                            ins=[],
                            outs=[],
                            sync_info=mybir.SyncInfo(on_wait=chunk, on_update=[]),
                        )
                        new_insts.append(nop)
                    si.on_wait = keep
                    changed = True
                new_insts.append(inst)
            if changed:
                bb.instructions.clear()
                for i in new_insts:
                    bb.instructions.append(i)
    return n_split


def _strip_const_loads(nc):
    """Remove preamble loads of the const page when nothing reads it."""
    from concourse import mybir

    used = set()
    removed = 0
    for f in nc.m.functions:
        for bb in f.blocks:
            for inst in bb.instructions:
                for arg in list(inst.ins):
                    t = getattr(getattr(arg, "bass_ap", None), "tensor", None)
                    n = getattr(t, "name", "") or ""
                    if n.startswith("const-"):
                        used.add(n)
    if used:
        return 0
    for f in nc.m.functions:
        for bb in f.blocks:
            keep = []
            for inst in bb.instructions:
                if type(inst).__name__ == "InstTensorLoad":
                    outs = list(inst.outs)
                    names = []
                    for a in outs:
                        t = getattr(getattr(a, "bass_ap", None), "tensor", None)
                        names.append(getattr(t, "name", "") or "")
                    if names and all(n.startswith("const-") for n in names):
                        removed += 1
                        continue
                keep.append(inst)
            if len(keep) != len(bb.instructions):
                bb.instructions.clear()
                for i in keep:
                    bb.instructions.append(i)
    return removed


def _strip_tail(nc):
    """Remove the end-of-kernel barrier/sem-reset tail.

    The runtime clears semaphores in its own exec preamble, and every
    engine's results flow into the output DMA via data-dependency
    semaphores, so the only thing that must remain is the sync-engine
    DRAIN that flushes the output DMA queue."""
    from concourse import mybir

    f = nc.m.functions[0]
    bb = f.blocks[-1]
    insts = list(bb.instructions)
    idx = None
    for i, inst in enumerate(insts):
        if isinstance(inst, mybir.InstDrain) and inst.engine == mybir.EngineType.SP:
            idx = i
            break
    if idx is None:
        return 0
    kept = insts[: idx + 1]
    drain = kept[-1]
    if drain.sync_info is not None:
        drain.sync_info.on_wait = []
    removed = len(insts) - len(kept)
    bb.instructions.clear()
    for i in kept:
        bb.instructions.append(i)
    return removed


def _np_sto_dtype(mode):
    if mode == "bf16":
        import ml_dtypes

        return ml_dtypes.bfloat16
    return np.float32


def _route(command):
    """Group sample indices by expert, pad each group to a multiple of
    8*128 and split evenly across cores.

    Returns caps [E] (rows per core per expert) and I [NCORES, R] row
    index arrays (R = sum(caps))."""
    caps = []
    parts = []  # per expert: [NCORES, cap_e] padded index array
    for e in range(NUM_COMMANDS):
        idx = np.nonzero(command == e)[0].astype(np.int64)
        n = len(idx)
        cap = int(np.ceil(n / (NCORES * P))) * P if n else 0
        caps.append(cap)
        if cap == 0:
            parts.append(np.zeros((NCORES, 0), np.int64))
            continue
        pad = NCORES * cap - n
        idx_pad = np.concatenate([idx, np.full(pad, idx[-1], np.int64)])
        parts.append(idx_pad.reshape(NCORES, cap))
    desc = sorted(range(NUM_COMMANDS), key=lambda e: -caps[e])
    # small expert first (fast DMA start), small expert last (short tail)
    order = [desc[2], desc[0], desc[1], desc[3]]
    I = [np.concatenate([parts[e][k] for e in order]) for k in range(NCORES)]
    return [caps[e] for e in order], order, np.stack(I)


def _build_program(R, caps, eorder, b2c, n_pos, mode):
    from contextlib import ExitStack

    import concourse.bass as bass
    import concourse.tile as tile
    from concourse import mybir

    f32 = mybir.dt.float32
    # matmul-operand dtype (the whole producer chain must carry it for
    # the fp32r BIR verifier) and elementwise/storage dtype
    if mode == "bf16":
        MMD = mybir.dt.bfloat16
        STO = mybir.dt.bfloat16
    elif mode == "f32r":
        MMD = mybir.dt.float32r
        STO = f32
    else:
        MMD = f32
        STO = f32
    T = R // P

    pack = os.environ.get("KERNEL_PACK_MEAS", "1") == "1"
    nc = bass.Bass()
    # all arrays are PRE-TILED on the host so every DMA is a dense
    # [partition, contiguous-bytes] copy (cheap descriptor generation)
    imgT_d = nc.declare_dram_parameter("img_pre", [P, 4 * R], MMD, isOutput=False)
    measT_d = nc.declare_dram_parameter(
        "measAug", [NUM_MEAS + 1, R], MMD, isOutput=False
    )
    A_d = nc.declare_dram_parameter("A_pre", [NUM_COMMANDS, P, 4 * EMB], MMD, isOutput=False)
    WfAug_d = nc.declare_dram_parameter(
        "WfAug_pre", [NUM_MEAS + 1, NUM_COMMANDS, EMB], MMD, isOutput=False
    )
    b2tail_d = nc.declare_dram_parameter("b2tail", [P, T], f32, isOutput=False)
    outp_d = nc.declare_dram_parameter("outp", [P, 2, T], f32, isOutput=True)

    with tile.TileContext(nc) as tc:
        with ExitStack() as ctx:
            const_pool = ctx.enter_context(tc.tile_pool(name="const", bufs=1))
            w_pool = ctx.enter_context(tc.tile_pool(name="w", bufs=16))
            img_pool = ctx.enter_context(tc.tile_pool(name="img", bufs=16))
            junk_pool = ctx.enter_context(tc.tile_pool(name="junk", bufs=4))
            out_pool = ctx.enter_context(tc.tile_pool(name="out", bufs=1))
            ps_pool = ctx.enter_context(tc.tile_pool(name="ps", bufs=6, space="PSUM"))
            psw_pool = ctx.enter_context(tc.tile_pool(name="psw", bufs=1, space="PSUM"))

            # greedy least-loaded DMA queue assignment over the three
            # DMA-capable engines (SP + ACT hwdge, Pool swdge), with
            # transfers grouped into waves chained by semaphores so the
            # first-needed expert's data doesn't share DMA bandwidth
            # with later experts' transfers
            from concourse.tile_rust import add_dep_helper

            dma_engines = [nc.sync, nc.scalar, nc.gpsimd]
            # measured queue service rates differ: sync-HW ~1.4x scalar-HW,
            # gpsimd-SW slightly slower; balance by completion time
            dma_speed = [1.4, 1.0, 0.92]
            dma_load = [0.0, 0.0, 0.0]
            waves = [[]]

            def dma(dst, src, nbytes):
                qi = dma_load.index(min(dma_load))
                dma_load[qi] += nbytes / dma_speed[qi]
                inst = dma_engines[qi].dma_start(dst, src)
                waves[-1].append(inst)
                return inst

            def next_wave():
                if waves[-1]:
                    waves.append([])

            esz = 2 if mode == "bf16" else 4
            mrows = P if pack else NUM_MEAS + 1
            nrep = 4 if pack else 1
            measT_sb = const_pool.tile([mrows, R], MMD)
            WfAug_sb = const_pool.tile([mrows, NUM_COMMANDS, EMB], MMD)
            for j in range(nrep):
                dma(
                    measT_sb[32 * j : 32 * j + NUM_MEAS + 1, :],
                    measT_d[:],
                    9 * R * esz,
                )
                dma(
                    WfAug_sb[32 * j : 32 * j + NUM_MEAS + 1, :, :],
                    WfAug_d[:],
                    9 * 4 * EMB * esz,
                )
            b2tail_sb = const_pool.tile([P, T], f32, tag="b2tail", name="b2tail_sb")
            dma(b2tail_sb[:], b2tail_d[:], P * T * 4)
            zbias = const_pool.tile([P, 1], f32)
            nc.vector.memset(zbias[:], 0.0)
            p_pos = {}
            p_neg = {}
            for i, cap in enumerate(caps):
                if cap == 0:
                    continue
                tseg = cap // P
                p_pos[i] = out_pool.tile([P, tseg], f32, tag=f"pp_{i}", name=f"pp_{i}")
                p_neg[i] = out_pool.tile([P, tseg], f32, tag=f"pn_{i}", name=f"pn_{i}")
                nc.vector.memset(p_pos[i][:], 0.0)
                nc.vector.memset(p_neg[i][:], 0.0)

            A_sb = {}
            img_sb = {}
            for i, cap in enumerate(caps):
                if cap == 0:
                    continue
                next_wave()
                e = eorder[i]
                base = 4 * sum(caps[:i])
                for c in range(4):
                    A_sb[i, c] = w_pool.tile(
                        [P, EMB], MMD, tag="A", name=f"A_sb_{i}_{c}"
                    )
                    dma(
                        A_sb[i, c][:],
                        A_d[e][:, c * EMB : (c + 1) * EMB],
                        P * EMB * esz,
                    )
                    img_sb[i, c] = img_pool.tile(
                        [P, cap], MMD, tag="img", name=f"img_sb_{i}_{c}"
                    )
                    dma(
                        img_sb[i, c][:],
                        imgT_d[:, base + c * cap : base + (c + 1) * cap],
                        P * cap * esz,
                    )

            if os.environ.get("KERNEL_WAVES", "0") == "1":
                for k in range(1, len(waves)):
                    gate = waves[k - 1][-1]
                    seen_eng = set()
                    for inst in waves[k]:
                        eng = inst.ins.engine
                        if eng in seen_eng:
                            continue
                        seen_eng.add(eng)
                        add_dep_helper(
                            inst.ins, gate.ins, sync=True, reason="dma wave chaining"
                        )

            # keep the PE busy during the initial DMA window so the HAM
            # clock gate is warm when real matmuls start
            warm_a = const_pool.tile([P, EMB], MMD, tag="warm_a", name="warm_a")
            nc.vector.memset(warm_a[:], 0.0)
            ps_w = psw_pool.tile([P, EMB], f32, tag="warm_ps", name="ps_warm")
            N_WARM = 16
            for w in range(N_WARM):
                nc.tensor.matmul(
                    ps_w[:],
                    lhsT=warm_a[:, :P],
                    rhs=warm_a[:],
                    start=(w == 0),
                    stop=(w == N_WARM - 1),
                )
            junkw = junk_pool.tile([P, EMB], STO, tag="junk")
            nc.scalar.activation(
                junkw[:], ps_w[:], mybir.ActivationFunctionType.Copy
            )

            for i, cap in enumerate(caps):
                e = eorder[i]
                off = sum(caps[:i])
                nt = cap // P
                group, ps_of = {}, {}
                for r0 in range(0, nt, 4):
                    group[r0] = list(range(r0, min(r0 + 4, nt)))
                for r in range(nt):
                    if pack and r in group:
                        # packed meas matmuls FIRST (start=True, concurrent
                        # row-groups), then each tile's img matmuls; the
                        # per-tile accum follows its own last img matmul
                        for j, rr in enumerate(group[r]):
                            psr = ps_pool.tile(
                                [P, EMB], f32, tag="h", name=f"ps_{i}_{rr}"
                            )
                            ps_of[rr] = psr
                            col = off + rr * P
                            nc.tensor.matmul(
                                psr[:],
                                lhsT=measT_sb[
                                    32 * j : 32 * j + NUM_MEAS + 1, col : col + P
                                ],
                                rhs=WfAug_sb[32 * j : 32 * j + NUM_MEAS + 1, e, :],
                                start=True,
                                stop=False,
                                tile_position=(32 * j, 0),
                            )
                        for rr in group[r]:
                            for ko in range(4):
                                nc.tensor.matmul(
                                    ps_of[rr][:],
                                    lhsT=img_sb[i, ko][:, rr * P : (rr + 1) * P],
                                    rhs=A_sb[i, ko][:],
                                    start=False,
                                    stop=(ko == 3),
                                )
                    if pack:
                        ps = ps_of[r]
                    else:
                        ps = ps_pool.tile([P, EMB], f32, tag="h")
                        for ko in range(4):
                            nc.tensor.matmul(
                                ps[:],
                                lhsT=img_sb[i, ko][:, r * P : (r + 1) * P],
                                rhs=A_sb[i, ko][:],
                                start=(ko == 0),
                                stop=False,
                            )
                        col = off + r * P
                        nc.tensor.matmul(
                            ps[:],
                            lhsT=measT_sb[:, col : col + P],
                            rhs=WfAug_sb[:, e, :],
                            start=False,
                            stop=True,
                        )
                    junk = junk_pool.tile([P, EMB], STO, tag="junk")
                    npe = n_pos[e]
                    if npe > 0:
                        nc.scalar.activation(
                            junk[:, :npe],
                            ps[:, :npe],
                            mybir.ActivationFunctionType.Relu,
                            bias=zbias[:],
                            accum_out=p_pos[i][:, r : r + 1],
                        )
                    if npe < EMB:
                        junk2 = junk_pool.tile([P, EMB], STO, tag="junk2")
                        nc.vector.tensor_scalar(
                            junk2[:, npe:],
                            ps[:, npe:],
                            0.0,
                            0.0,
                            mybir.AluOpType.max,
                            mybir.AluOpType.add,
                            accum_out=p_neg[i][:, r : r + 1],
                        )

                tseg = cap // P
                seg = slice(off // P, off // P + tseg)
                q = out_pool.tile([P, tseg], f32, tag=f"q_{i}", name=f"q_{i}")
                sig = out_pool.tile([P, tseg], f32, tag=f"sig_{i}", name=f"sig_{i}")
                outs = out_pool.tile(
                    [P, 2, tseg], f32, tag=f"outs_{i}", name=f"outs_{i}"
                )
                nc.vector.tensor_tensor(
                    q[:], p_pos[i][:], p_neg[i][:], mybir.AluOpType.subtract
                )
                nc.vector.tensor_add(q[:], q[:], b2tail_sb[:, seg])
                nc.scalar.activation(
                    sig[:],
                    q[:],
                    mybir.ActivationFunctionType.Sigmoid,
                    bias=zbias[:],
                )
                nc.vector.tensor_scalar_mul(outs[:, 0, :], sig[:], 50.0)
                nc.vector.tensor_scalar(
                    outs[:, 1, :],
                    q[:],
                    1.0,
                    -1.0,
                    mybir.AluOpType.min,
                    mybir.AluOpType.max,
                )
                dma(outp_d[:, :, seg], outs[:], P * 2 * tseg * 4)



    _strip_tail(nc)
    _split_excess_waits(nc)
    return nc


def _prepare(inputs, mode):
    img_embs = np.asarray(inputs["img_embs"], np.float32)
    measurements = np.asarray(inputs["measurements"], np.float32)
    command = np.asarray(inputs["command"])
    W_meas = np.asarray(inputs["W_meas"], np.float32)
    b_meas = np.asarray(inputs["b_meas"], np.float32)
    W1 = np.asarray(inputs["W1"], np.float32)
    b1 = np.asarray(inputs["b1"], np.float32)
    W2 = np.asarray(inputs["W2"], np.float32)
    b2 = np.asarray(inputs["b2"], np.float32)

    sto = _np_sto_dtype(mode)
    caps, eorder, I = _route(command)
    R = int(sum(caps))

    # fold measurement path (float64 for the host-side precompute)
    W1h = W1[:, EMB:, :].astype(np.float64)
    Wf = np.einsum("md,edh->emh", W_meas.astype(np.float64), W1h)
    b_eff = np.einsum("d,edh->eh", b_meas.astype(np.float64), W1h) + b1
    A64 = W1[:, :EMB, :].astype(np.float64)

    # fold |w2[:, 0]| into the hidden columns and permute them so the
    # w2>0 columns come first: p = sum(relu(pos cols)) - sum(relu(neg
    # cols)), computed for free by the ACT accum during the relu pass.
    w2c = W2[:, :, 0].astype(np.float64)
    n_pos = []
    A_s = np.empty_like(A64)
    Wf_s = np.empty_like(Wf)
    b_eff_s = np.empty_like(b_eff)
    for e in range(NUM_COMMANDS):
        perm = np.argsort(w2c[e] <= 0, kind="stable")
        n_pos.append(int((w2c[e] > 0).sum()))
        sc = np.abs(w2c[e])[perm]
        A_s[e] = A64[e][:, perm] * sc[None, :]
        Wf_s[e] = Wf[e][:, perm] * sc[None, :]
        b_eff_s[e] = b_eff[e][perm] * sc
    WfAug = np.concatenate([Wf_s, b_eff_s[:, None, :]], axis=1).astype(sto)
    A = np.ascontiguousarray(A_s).astype(sto)  # [E,512,512]
    b2c = [float(x) for x in b2[:, 0]]

    T = R // P
    col_expert = np.concatenate(
        [np.full(caps[i] // P, eorder[i], np.int64) for i in range(NUM_COMMANDS)]
    )
    b2tail = np.broadcast_to(
        np.array([b2c[e] for e in col_expert], np.float32)[None, :], (P, T)
    ).copy()

    # pre-tiled shared weights: every device DMA is a dense 2D copy
    A_pre = np.ascontiguousarray(
        A.reshape(NUM_COMMANDS, 4, P, EMB).transpose(0, 2, 1, 3).reshape(
            NUM_COMMANDS, P, 4 * EMB
        )
    )
    WfAug_pre = np.ascontiguousarray(WfAug.transpose(1, 0, 2))  # [9, E, 512]

    imgT = img_embs.T.astype(sto)  # [512, B] cast once
    measT = measurements.T  # [8, B]
    ones_row = np.ones((1, R), np.float32).astype(sto)
    in_maps = []
    for k in range(NCORES):
        Ik = I[k]
        imgT_k = imgT[:, Ik].reshape(4, P, R)  # [o, p, r]
        img_pre = np.concatenate(
            [
                imgT_k[:, :, sum(caps[:e]) : sum(caps[: e + 1])]
                .transpose(1, 0, 2)
                .reshape(P, 4 * caps[e])
                for e in range(NUM_COMMANDS)
                if caps[e]
            ],
            axis=1,
        )
        measAug_k = np.concatenate(
            [measT[:, Ik].astype(sto), ones_row], axis=0
        )
        in_maps.append(
            {
                "img_pre": np.ascontiguousarray(img_pre),
                "measAug": measAug_k,
                "A_pre": A_pre,
                "WfAug_pre": WfAug_pre,
                "b2tail": b2tail,
            }
        )
    return in_maps, I, R, caps, eorder, b2c, n_pos


def _run(inputs, mode=None, trace=False):
    """Returns ((angle, speed), BassKernelResults)."""
    mode = mode or MODE
    _install_ntff_shim()
    from concourse.bass_utils import run_bass_kernel_spmd

    in_maps, I, R, caps, eorder, b2c, n_pos = _prepare(inputs, mode)
    key = (
        R,
        tuple(caps),
        tuple(eorder),
        mode,
        tuple(np.float32(b) for b in b2c),
        tuple(n_pos),
    )
    if key not in _CACHE:
        _CACHE[key] = _build_program(R, caps, eorder, b2c, n_pos, mode)
    nc = _CACHE[key]

    res = run_bass_kernel_spmd(
        nc, in_maps, core_ids=list(range(NCORES)), trace=trace
    )

    nb = int(np.asarray(inputs["command"]).shape[0])
    angle = np.zeros(nb, np.float32)
    speed = np.zeros(nb, np.float32)
    for k in range(NCORES):
        outp = res.results[k]["outp"]  # [128, 2, T]
        Ik = I[k]
        angle[Ik] = outp[:, 0, :].T.reshape(R)
        speed[Ik] = outp[:, 1, :].T.reshape(R)
    return (angle, speed), res


def kernel(**inputs):
    out, _ = _run(inputs)
    return out
